# revision 52
# baseline (speedup 1.0000x reference)
"""Multi-head attention (B=4, S=2048, E=1024, H=16, D=64) on 8 TRN2 cores.

Sharding: heads 2c, 2c+1 on core c (Megatron-style column-parallel qkv,
row-parallel out-projection; partial outputs summed on host).

Per-core dataflow (v4):
  A) qkv projection in bf16 (host-cast xT/Wqkv). 8 accumulation steps of
     K=128. q,k stored bf16 [128,2,S] with head h on partitions
     h*64..h*64+63; v stored f32r.
  B) scores via K=128 zero-padded bf16 matmuls. BOTH heads' scores for
     one kt land in one [128,2,512] PSUM tile (2 banks) that FILLS IN
     ONE KT, then ONE ScalarE exp per kt (N=1024, (N+352)/1.2ns)
     evacuates PSUM and computes in a single op -- ScalarE is the only
     cheap PSUM evacuation path (DVE PSUM-fp32 runs 1x: a staging-copy
     variant measured 145us SLOWER). The sc ring is 3 deep and shared
     with feed psum; tile lifetime ~1.8kt < ring rotation, so the
     in-order PE queue never waits on exp draining a bank.
  C) at-matmuls (probs @ v, bf16 [128,65] with ones-column denominator
     in row 64 of the [65,512] accumulators) and chunk normalizations
     are deferred through a WORK QUEUE popped 2/kt with a 6-entry
     backlog (~3kt, hiding the scores->exp round-trip) that spills
     across chunk boundaries -- chunk c's tail at-matmuls and norm run
     inside chunk c+1's kt loop, so there is no serial flush at group
     ends. PSUM: sc ring 3x2 banks + at 2x1 = 8 banks.
  D) row-parallel out-projection in bf16 (FWL weight loads; f32r
     weights measured +60us of unhidden 4-byte LDWEIGHTS). half-o-tile
     units [128,1024], all half=0 units before half=1 so the ab reads
     of chunks 2-3 (whose norms spill into the next batch) come late.
     yT written bf16, host sums partials in f32.

Projection work is injected INTO the attention kt-loop as "feed units"
(qkv m-chunks of batch b+1, half-o-tile out-projections of batch b-1),
one per kt. The batch indices cycle mod B with persistent ab storage,
software-pipelining across For_i iterations (prologue before the loop,
flush after).
"""
from contextlib import ExitStack

import numpy as np
import ml_dtypes

import concourse.bass as bass
import concourse.mybir as mybir
import concourse.tile as tile
from concourse import bacc
from concourse.bass_utils import run_bass_kernel_spmd
from concourse.masks import make_identity

B, S, E, H, D = 4, 2048, 1024, 16, 64
NCORES = 8
HPC = H // NCORES        # 2 heads per core
F = HPC * D              # 128 local features
M3 = 3 * F               # 384 local qkv rows
BS = B * S               # 8192
KT_E = E // 128          # 8 contraction steps for projections
KT_S = S // 128          # 16 sk tiles
CW = 512                 # attention chunk width (sq per group)
NCH = S // CW            # 4 chunks per batch
f32 = mybir.dt.float32
f32r = mybir.dt.float32r
bf16 = mybir.dt.bfloat16
EXP = mybir.ActivationFunctionType.Exp
EXP_SCALE = 0.125            # 1/sqrt(D)

# workq backlog: deferral (in queue entries, 2/kt) hiding the
# scores->exp round-trip from the in-order PE queue
BACKLOG = 6

_prog_cache = {}


def build_program(niter=None, parts="Aao"):
    """niter=None: normal external-I/O program. niter=N: timing variant with
    internal DRAM x/y and the whole body in a device-side For_i loop."""
    key = ("nc", niter, parts, BACKLOG)
    if key in _prog_cache:
        return _prog_cache[key]
    nc = bacc.Bacc("TRN2", target_bir_lowering=False)
    if niter is None:
        xT = nc.dram_tensor("xT", [E, BS], bf16, kind="ExternalInput")
        yT = nc.dram_tensor("yT", [E, BS], bf16, kind="ExternalOutput")
    else:
        xT = nc.dram_tensor("xTi", [E, BS], bf16, kind="Internal")
        yT = nc.dram_tensor("yTi", [E, BS], bf16, kind="Internal")
    wq = nc.dram_tensor("wq", [E, M3], bf16, kind="ExternalInput")
    bq = nc.dram_tensor("bq", [128, 3], f32, kind="ExternalInput")
    wo = nc.dram_tensor("wo", [F, E], bf16, kind="ExternalInput")
    bo = nc.dram_tensor("bo", [128, E // 128], f32, kind="ExternalInput")
    if niter is not None:
        tout = nc.dram_tensor("tout", [1, 3], f32, kind="ExternalOutput")

    with tile.TileContext(nc) as tc, ExitStack() as ctx:
        const = ctx.enter_context(tc.tile_pool(name="const", bufs=1))
        xp = ctx.enter_context(tc.tile_pool(name="xp", bufs=4))
        expp = ctx.enter_context(tc.tile_pool(name="expp", bufs=12))
        anp = ctx.enter_context(tc.tile_pool(name="anp", bufs=12))
        ystp = ctx.enter_context(tc.tile_pool(name="ystp", bufs=8))
        # PSUM budget (8 banks):
        #   pssc "sc" 3x [128,2,512] (2 banks each): per-kt both-heads
        #     scores tiles, feed psum, v transposes
        #   psat "at" 2x [65,512] accumulators (1 bank each)
        pssc = ctx.enter_context(tc.tile_pool(name="pssc", bufs=3, space="PSUM"))
        psat = ctx.enter_context(tc.tile_pool(name="psat", bufs=2, space="PSUM"))

        wq_sb = const.tile([128, KT_E, M3], bf16)
        nc.gpsimd.dma_start(
            out=wq_sb, in_=wq.rearrange("(kt p) m -> p kt m", p=128))
        wo_sb = const.tile([F, E], bf16)
        nc.gpsimd.dma_start(out=wo_sb, in_=wo[:, :])
        bq_sb = const.tile([128, 3], f32)
        nc.gpsimd.dma_start(out=bq_sb, in_=bq[:, :])
        bo_sb = const.tile([128, E // 128], f32)
        nc.gpsimd.dma_start(out=bo_sb, in_=bo[:, :])
        id_f32 = const.tile([128, 128], f32)
        make_identity(nc, id_f32)
        id_sb = const.tile([128, 128], f32r)
        nc.vector.tensor_copy(id_sb, id_f32)

        xT_r = xT.rearrange("(kt p) n -> p kt n", p=128)

        # persistent double-buffered qkv storage, slot b%2. q is stored
        # zero-padded per head (q0: head0 rows live, head1 rows zero; q1
        # vice-versa) so one k-tile LDWEIGHTS serves both heads' scores.
        q0_st = const.tile([128, 2, S], bf16, name="q0_st")
        q1_st = const.tile([128, 2, S], bf16, name="q1_st")
        nc.vector.memset(q0_st[64:128, :, :], 0.0)
        nc.vector.memset(q1_st[0:64, :, :], 0.0)
        k_st = const.tile([128, 2, S], bf16, name="k_st")
        v_st = const.tile([128, 2, S], f32r, name="v_st")
        # vk: per (slot, head*kt) an 80-col block; cols 0..63 = vT,
        # col 64 = ones (denominator row)
        vk_st = const.tile([128, 2, HPC * KT_S, 80], bf16, name="vk_st")
        nc.vector.memset(vk_st[:, :, :, 64:65], 1.0)
        # persistent attention-output storage (slot b%2) so out-projection
        # of batch 3 can be fed into the NEXT loop iteration's batch-0
        # groups (software pipelining across For_i iterations). bf16 so
        # the out-projection runs bf16 x bf16 (FWL weight loads).
        ab_st = const.tile([128, 2, S], bf16, name="ab_st")

        excons = (const.tile([1, 4], f32, name="excons")
                  if "t" in parts else None)

        # deferred work: at-matmul and normalization closures, popped
        # 2/kt inside every chunk's kt loop (spills across chunks)
        workq = []

        def pop_work(n, backlog=0):
            """Pop up to n closures, keeping `backlog` entries queued --
            the backlog is the deferral that hides the scores->exp
            round-trip from the in-order PE queue."""
            for _ in range(n):
                if len(workq) > backlog:
                    workq.pop(0)()

        def emit_A_chunk(n):
            b, nl = divmod(n, 4)
            sl = b % 2
            cs = slice(nl * 512, (nl + 1) * 512)
            xc = xp.tile([128, KT_E, 512], bf16, tag="xc")
            nc.sync.dma_start(out=xc, in_=xT_r[:, :, n * 512:(n + 1) * 512])
            for m in range(3):
                ps = pssc.tile([128, 512], f32, tag="sc")
                for kt in range(KT_E):
                    nc.tensor.matmul(
                        ps, lhsT=wq_sb[:, kt, m * 128:(m + 1) * 128],
                        rhs=xc[:, kt, :],
                        start=(kt == 0), stop=(kt == KT_E - 1))
                if m == 0:
                    nc.vector.tensor_scalar_add(
                        q0_st[0:64, sl, cs], ps[0:64, :], bq_sb[0:64, 0:1])
                    nc.vector.tensor_scalar_add(
                        q1_st[64:128, sl, cs], ps[64:128, :],
                        bq_sb[64:128, 0:1])
                else:
                    dst = (None, k_st, v_st)[m]
                    nc.vector.tensor_scalar_add(
                        dst[:, sl, cs], ps, bq_sb[:, m:m + 1])

        # per-batch vk AP dicts, filled by vt feed units (emitted during
        # the PREVIOUS batch, right after the m-chunk producing their v
        # slice). The APs are fixed offsets into vk_st, so re-emission
        # for the same slot yields identical APs.
        vkd = {bb: {} for bb in range(B)}

        def emit_vt(b, kt):
            """One full 128x128 transpose covers both heads' v."""
            sl = b % 2
            vt = pssc.tile([128, 128], f32r, tag="sc")
            nc.tensor.transpose(
                vt, in_=v_st[:, sl, kt * 128:(kt + 1) * 128],
                identity=id_sb)
            for h in range(HPC):
                j = h * KT_S + kt
                nc.vector.tensor_copy(
                    vk_st[:, sl, j, 0:64], vt[:, h * 64:(h + 1) * 64])
                vkd[b][(h, kt)] = vk_st[:, sl, j, 0:65]

        def emit_norm(b, c, h, at_h, ab):
            cq = c * CW
            rs = anp.tile([65, CW], f32, tag="norm")
            nc.vector.reciprocal(rs[64:65, :], at_h[64:65, :])
            nc.sync.dma_start(out=rs[0:1, :], in_=rs[64:65, :])
            rb = anp.tile([64, CW], f32, tag="norm")
            nc.gpsimd.partition_broadcast(rb, rs[0:1, :])
            if h == 0:
                nc.vector.tensor_mul(
                    ab[0:64, cq:cq + CW], at_h[0:64, :], rb)
            else:
                nm = anp.tile([64, CW], bf16, tag="norm")
                nc.vector.tensor_mul(nm, at_h[0:64, :], rb)
                nc.sync.dma_start(
                    out=ab[64:128, cq:cq + CW], in_=nm)

        def emit_attn_group(b, c, vk, ab, feeds=()):
            """Both heads for sq chunk c (CW=512 wide). Per kt: 2 score
            matmuls into one [128,2,512] PSUM tile, 1 exp (N=1024) into
            an SBUF [128,2,512] bf16 tile; at-matmuls and the chunk norm
            go through workq (2 pops/kt, 6-entry backlog, spilling into
            the next chunk). One feed unit runs per kt."""
            feeds = list(feeds)
            skip_at = "t" in parts
            cq = c * CW
            sl = b % 2
            at = [] if skip_at else [
                psat.tile([65, CW], f32, tag="at", name=f"at{b}{c}{h}")
                for h in range(HPC)]

            def at_mm(kt, h, ex):
                def f():
                    nc.tensor.matmul(
                        at[h], lhsT=vk[(h, kt)], rhs=ex,
                        start=(kt == 0), stop=(kt == KT_S - 1))
                return f

            for kt in range(KT_S):
                ko = kt * 128
                # pop deferred work around the scores: its deps are ~3kt
                # old, so it is the least likely to stall the in-order
                # PE queue; splitting the pops smooths the per-kt mix
                pop_work(1, backlog=BACKLOG)
                sc = pssc.tile([128, HPC, CW], f32, tag="sc")
                for h, qz in ((0, q0_st), (1, q1_st)):
                    nc.tensor.matmul(
                        sc[:, h, :],
                        lhsT=k_st[:, sl, ko:ko + 128],
                        rhs=qz[:, sl, cq:cq + CW],
                        start=True, stop=True)
                ex = expp.tile([128, HPC, CW], bf16, tag="exp")
                nc.scalar.activation(ex, sc, EXP, scale=EXP_SCALE)
                pop_work(1, backlog=BACKLOG)
                if skip_at:
                    nc.vector.tensor_copy(
                        excons, ex[0:1, 0, 0:8].bitcast(f32))
                else:
                    for h in range(HPC):
                        workq.append(at_mm(kt, h, ex[:, h, :]))
                if feeds:
                    feeds.pop(0)()
            if skip_at:
                while feeds:
                    feeds.pop(0)()
                return
            for h in range(HPC):
                workq.append(
                    (lambda hh, att: lambda: emit_norm(b, c, hh, att, ab))(
                        h, at[h]))
            while feeds:
                feeds.pop(0)()

        def A_chunk_units(n):
            """Split one A-chunk into 8 feed units: DMA + 3 m-chunks +
            4 v-transposes (after m2, which produces their v slice)."""
            b, nl = divmod(n, 4)
            cs = slice(nl * 512, (nl + 1) * 512)
            box = {}

            def dma_unit():
                xc = xp.tile([128, KT_E, 512], bf16, tag="xc")
                nc.sync.dma_start(
                    out=xc, in_=xT_r[:, :, n * 512:(n + 1) * 512])
                box["xc"] = xc
                return 0

            def m_unit(m):
                def f():
                    ps = pssc.tile([128, 512], f32, tag="sc")
                    for kt in range(KT_E):
                        nc.tensor.matmul(
                            ps, lhsT=wq_sb[:, kt, m * 128:(m + 1) * 128],
                            rhs=box["xc"][:, kt, :],
                            start=(kt == 0), stop=(kt == KT_E - 1))
                    sl = b % 2
                    if m == 0:
                        nc.vector.tensor_scalar_add(
                            q0_st[0:64, sl, cs], ps[0:64, :],
                            bq_sb[0:64, 0:1])
                        nc.vector.tensor_scalar_add(
                            q1_st[64:128, sl, cs], ps[64:128, :],
                            bq_sb[64:128, 0:1])
                    else:
                        dst = (None, k_st, v_st)[m]
                        nc.vector.tensor_scalar_add(
                            dst[:, sl, cs], ps, bq_sb[:, m:m + 1])
                    return 1
                return f

            def vt_unit(kt):
                def f():
                    emit_vt(b, kt)
                    return 1
                return f

            return ([dma_unit, m_unit(0), m_unit(1), m_unit(2)]
                    + [vt_unit(4 * nl + j) for j in range(4)])

        def outproj_units(b, ab):
            """16 feed units per batch: one [128,1024] yp half-o-tile
            each (2 mms + 1 FD-1024 bias-add; DMA on the 2nd half).
            All half=0 units precede half=1 so the reads of ab chunks
            2-3 (whose norms spill into this batch) come late."""
            ysts = {}
            units = []
            for half in range(2):
                for o in range(8):
                    def mk(o, half):
                        def f():
                            if half == 0:
                                ysts[o] = ystp.tile(
                                    [128, S], bf16, tag="yst",
                                    name=f"yst{b}o{o}")
                            yst = ysts[o]
                            yp = pssc.tile([128, 1024], f32, tag="sc")
                            for i, c4 in enumerate((2 * half, 2 * half + 1)):
                                nc.tensor.matmul(
                                    yp[:, i * 512:(i + 1) * 512],
                                    lhsT=wo_sb[:, o * 128:(o + 1) * 128],
                                    rhs=ab[:, c4 * 512:(c4 + 1) * 512],
                                    start=True, stop=True)
                            nc.vector.tensor_scalar_add(
                                yst[:, half * 1024:(half + 1) * 1024], yp,
                                bo_sb[:, o:o + 1])
                            if half == 1:
                                nc.sync.dma_start(
                                    out=yT[o * 128:(o + 1) * 128,
                                           b * S:(b + 1) * S],
                                    in_=yst)
                            return 1
                        return f
                    units.append(mk(o, half))
            return units

        def interleave(a, bls):
            out = []
            for i in range(max(len(a), len(bls))):
                if i < len(a):
                    out.append(a[i])
                if i < len(bls):
                    out.append(bls[i])
            return out

        def body():
            abs_ = {bb: ab_st[:, bb % 2, :] for bb in range(B)}
            for b in range(B):
                au = []
                for n4 in range(4):
                    au += A_chunk_units((4 * (b + 1) + n4) % 16)
                ou = []
                if "o" in parts:
                    ou = outproj_units((b - 1) % B, abs_[(b - 1) % B])
                feeds = interleave(au, ou)
                if "a" in parts and "t" not in parts:
                    nf = len(feeds)
                    for c in range(NCH):
                        emit_attn_group(
                            b, c, vkd[b], abs_[b],
                            feeds[c * nf // NCH:(c + 1) * nf // NCH])
                else:
                    if "a" in parts:
                        for c in range(NCH):
                            emit_attn_group(b, c, vkd[b], abs_[b])
                    for f in feeds:
                        f()
                if niter is not None and parts != "Aao" and "o" not in parts:
                    cons_b = const.tile([1, 4], f32, name=f"cons{b}", bufs=1) \
                        if b == 0 else cons_b
                    nc.vector.tensor_copy(
                        cons_b, v_st[0:1, b % 2, 0:4].bitcast(f32))
                    for t in (q0_st, q1_st, k_st):
                        nc.vector.tensor_copy(cons_b, t[0:1, b % 2, 0:4])
                    if "a" in parts and "t" not in parts:
                        while workq:
                            pop_work(1)
                        nc.vector.tensor_copy(
                            cons_b, abs_[b][0:1, 0:8].bitcast(f32))
            # drain deferred at-matmuls + norms before the loop repeats
            while workq:
                pop_work(1)

        def prologue():
            # batch 0's A chunks + v transposes; in the For_i steady
            # state these are produced by the previous iteration's
            # batch-3 feeds.
            for n in range(4):
                emit_A_chunk(n)
            for kt in range(KT_S):
                emit_vt(0, kt)

        def flush():
            # final batch-3 out-projection (fed from batch-0 groups of the
            # next iteration in steady state; re-emitted here for the tail)
            if "o" in parts:
                for f in outproj_units(B - 1, ab_st[:, (B - 1) % 2, :]):
                    f()

        if niter is None:
            prologue()
            body()
            flush()
        else:
            prologue()
            with tc.For_i(0, niter, 1):
                body()
            flush()
            dmy = const.tile([1, 3], f32)
            nc.vector.tensor_copy(dmy, bq_sb[0:1, 0:3])
            nc.gpsimd.dma_start(out=tout[:, :], in_=dmy)

    nc.compile()
    _prog_cache[key] = nc
    return nc


def make_in_maps(x, W_qkv, b_qkv, W_out, b_out):
    xTb = np.ascontiguousarray(x.reshape(BS, E).T).astype(ml_dtypes.bfloat16)
    in_maps = []
    for c in range(NCORES):
        rows, brows = [], []
        for blk in range(3):
            for h in (HPC * c, HPC * c + 1):
                rows.append(W_qkv[blk * E + h * D: blk * E + (h + 1) * D, :])
                brows.append(b_qkv[blk * E + h * D: blk * E + (h + 1) * D])
        W_loc = np.concatenate(rows, axis=0)            # [384, 1024]
        b_loc = np.concatenate(brows, axis=0)           # [384]
        wq_in = np.ascontiguousarray(W_loc.T).astype(ml_dtypes.bfloat16)
        bq_in = np.ascontiguousarray(
            b_loc.reshape(3, 128).T).astype(np.float32)
        wo_in = np.ascontiguousarray(
            W_out[:, c * F:(c + 1) * F].T).astype(ml_dtypes.bfloat16)
        if c == 0:
            bo_in = np.ascontiguousarray(
                b_out.reshape(E // 128, 128).T).astype(np.float32)
        else:
            bo_in = np.zeros((128, E // 128), dtype=np.float32)
        in_maps.append(
            {"xT": xTb, "wq": wq_in, "bq": bq_in, "wo": wo_in, "bo": bo_in})
    return in_maps


def kernel(x, W_qkv, b_qkv, W_out, b_out):
    x = np.asarray(x, dtype=np.float32)
    W_qkv = np.asarray(W_qkv, dtype=np.float32)
    b_qkv = np.asarray(b_qkv, dtype=np.float32)
    W_out = np.asarray(W_out, dtype=np.float32)
    b_out = np.asarray(b_out, dtype=np.float32)

    nc = build_program()
    in_maps = make_in_maps(x, W_qkv, b_qkv, W_out, b_out)
    res = run_bass_kernel_spmd(nc, in_maps, core_ids=list(range(NCORES)))
    acc = np.zeros((E, BS), dtype=np.float32)
    for c in range(NCORES):
        acc += res.results[c]["yT"].astype(np.float32)
    return np.ascontiguousarray(acc.T).reshape(B, S, E)


if __name__ == "__main__":
    rng = np.random.default_rng(0)
    x = rng.standard_normal((B, S, E), dtype=np.float32)
    s = 1.0 / np.sqrt(E)
    W_qkv = rng.uniform(-s, s, (3 * E, E)).astype(np.float32)
    b_qkv = rng.uniform(-s, s, (3 * E,)).astype(np.float32)
    W_out = rng.uniform(-s, s, (E, E)).astype(np.float32)
    b_out = rng.uniform(-s, s, (E,)).astype(np.float32)
    y = kernel(x, W_qkv, b_qkv, W_out, b_out)
    print("out", y.shape, y.dtype, float(np.abs(y).max()))
